# revision 16
# baseline (speedup 1.0000x reference)
"""Trainium2 Bass kernel for per-element tiny MLPs (fp16, software-pipelined).

Problem: N=4,000,000 independent 1->8->1 MLPs:
    y[i] = W2[i] @ relu(W1[i] * x[i] + b1[i]) + b2[i]

Memory-bound + DVE-bound. Sharded over 8 NeuronCores by net index (data
parallel, no communication).

Design (vs the 179-215us fp32 baseline):
  * fp16 everywhere: halves HBM traffic (52B/net in, 2B out) and gives
    tensor_tensor the 2x_1p DVE perf mode (0.52ns/elem measured, vs
    1.04 for fp32). Host-side accuracy sim: rel_l2 ~ 5e-4 (budget 2e-2).
  * hidden-dim-OUTER device layout: a weight tile is [128, 8*f] with the
    hidden index j as the outer free-dim block, so the per-net segmented
    sum is a 3-step tree of CONTIGUOUS 2x-mode tensor_tensor adds
    (tensor_reduce has no fast mode: 1 elem/cyc).
  * software pipeline: per tile, phase A = {mult x*W1, add b1} and
    phase B = {mult *W2, 3-level tree, +b2}; emitted as A_t, B_{t-1} so
    the in-order DVE stream always has B-work of the previous tile while
    ACT runs relu_t. (Without this, DVE idled ~4.5us/tile waiting on
    relu: measured 137us wall.)
  * input streams split by consumer phase: one [128, 32f] DMA carries
    w1|b1 (phase A, bufs=3 for ~2 tiles of DMA lookahead), one [128,16f]
    carries w2 (phase B), one [128, 4f] carries x|b2.
  * relu stays on the otherwise-idle ACT engine; scalar_tensor_tensor
    would fuse relu+mult but runs at 1x (no fast uop): net loss.

Per-core budget: DVE ~75us busy (32 fp16 elem/net at 2x + ~165ns/op
x 56 ops, x ~1.15 DMA-contention), DMA 27MB at ~420GB/s peak ~ 64us,
ACT relu ~27us. GPSIMD/PE idle (gpsimd steals DVE SBUF ports; PE fp32
4cyc/row and PSUM results cost 1x-mode DVE post-ops).
"""

import numpy as np
from contextlib import ExitStack

import concourse.bacc as bacc
import concourse.mybir as mybir
import concourse.tile as tile
from concourse.bass_utils import run_bass_kernel_spmd

F16 = mybir.dt.float16
AF = mybir.ActivationFunctionType
OP = mybir.AluOpType

N = 4_000_000
H = 8
N_CORES = 8
R = N // N_CORES            # 500,000 nets per core
FP = 3908                   # free-dim cols per partition: 128*3908 = 500,224
R_PAD = 128 * FP
# Ramp-up, steady-state, ramp-down tile sizes (sum = FP, all even).
FIS = [16, 88, 256, 672, 672, 672, 672, 672, 188]
assert sum(FIS) == FP and all(f % 2 == 0 for f in FIS)


def build_nc(fis):
    fp = sum(fis)

    nc = bacc.Bacc("TRN2", target_bir_lowering=False, debug=False)

    wa = nc.dram_tensor("wa", [128, 16 * fp], F16, kind="ExternalInput")  # w1|b1
    wb = nc.dram_tensor("wb", [128, 8 * fp], F16, kind="ExternalInput")   # w2
    sm = nc.dram_tensor("sm", [128, 2 * fp], F16, kind="ExternalInput")   # x|b2
    ys = nc.dram_tensor("ys", [128, fp], F16, kind="ExternalOutput")

    with tile.TileContext(nc) as tc, ExitStack() as ctx:
        wpool = ctx.enter_context(tc.tile_pool(name="w", bufs=2))
        zpool = ctx.enter_context(tc.tile_pool(name="z", bufs=2))
        vpool = ctx.enter_context(tc.tile_pool(name="v", bufs=2))

        state = []    # tiles awaiting phase U (umults; need relu done)
        ready_r = []  # tiles awaiting phase R (reduce tree; need u)

        def phase_a(fi, nbf):
            st = vpool.tile([128, 2 * fi], F16, tag="st", bufs=3)
            nc.sync.dma_start(st[:], sm.ap()[:, 2 * nbf:2 * (nbf + fi)])
            w12 = wpool.tile([128, 16 * fi], F16, tag="w12", bufs=3)
            nc.sync.dma_start(w12[:], wa.ap()[:, 16 * nbf:16 * (nbf + fi)])
            w2t = wpool.tile([128, 8 * fi], F16, tag="w2t")
            nc.sync.dma_start(w2t[:], wb.ap()[:, 8 * nbf:8 * (nbf + fi)])

            w1 = w12[:][:, 0:8 * fi].rearrange("p (j f) -> p j f", j=H)
            b1 = w12[:][:, 8 * fi:16 * fi]
            xb = st[:][:, 0:fi].rearrange("p f -> p () f").broadcast_to([128, H, fi])

            za = zpool.tile([128, 8 * fi], F16, tag="za", bufs=3)
            nc.vector.tensor_tensor(
                za[:].rearrange("p (j f) -> p j f", j=H), xb, w1, op=OP.mult
            )
            # lo/hi halves (hidden j 0..3 / 4..7): fine-grained deps so the
            # relu halves on ACT overlap the DVE stream with <=half-relu
            # exposure regardless of scheduler order.
            zb = zpool.tile([128, 8 * fi], F16, tag="zb")
            nc.vector.tensor_tensor(zb[:][:, 0:4 * fi], za[:][:, 0:4 * fi],
                                    b1[:, 0:4 * fi], op=OP.add)
            nc.vector.tensor_tensor(zb[:][:, 4 * fi:8 * fi], za[:][:, 4 * fi:8 * fi],
                                    b1[:, 4 * fi:8 * fi], op=OP.add)
            hc = zpool.tile([128, 8 * fi], F16, tag="za", bufs=3)  # reuse ring
            for q in range(4):
                nc.scalar.activation(hc[:][:, 2 * q * fi:2 * (q + 1) * fi],
                                     zb[:][:, 2 * q * fi:2 * (q + 1) * fi], AF.Relu)
            state.append((fi, nbf, w2t, st, hc))

        def phase_u():
            fi, nbf, w2t, st, hc = state.pop(0)
            u = zpool.tile([128, 8 * fi], F16, tag="u")
            for q in range(4):
                sl = slice(2 * q * fi, 2 * (q + 1) * fi)
                nc.vector.tensor_tensor(u[:][:, sl], hc[:][:, sl],
                                        w2t[:][:, sl], op=OP.mult)
            ready_r.append((fi, nbf, st, u))

        def phase_r():
            fi, nbf, st, u = ready_r.pop(0)
            b2a = st[:][:, fi:2 * fi]
            r1 = vpool.tile([128, 4 * fi], F16, tag="r1")
            nc.vector.tensor_tensor(r1[:], u[:][:, 0:4 * fi],
                                    u[:][:, 4 * fi:8 * fi], op=OP.add)
            r2 = vpool.tile([128, 2 * fi], F16, tag="r2")
            nc.vector.tensor_tensor(r2[:], r1[:][:, 0:2 * fi],
                                    r1[:][:, 2 * fi:4 * fi], op=OP.add)
            r3 = vpool.tile([128, fi], F16, tag="r3")
            nc.vector.tensor_tensor(r3[:], r2[:][:, 0:fi], r2[:][:, fi:2 * fi],
                                    op=OP.add)
            yo = vpool.tile([128, fi], F16, tag="yo")
            nc.vector.tensor_tensor(yo[:], r3[:], b2a, op=OP.add)
            nc.gpsimd.dma_start(ys.ap()[:, nbf:nbf + fi], yo[:])

        # Emission order per block: A_t, U_{t-1}, R_{t-1} - the umults of
        # tile t-1 sit AFTER the next tile's A-phase in the DVE stream, so
        # relu_{t-1} has a full A-phase (plus the prior R) of cover before
        # anything waits on it.
        nbf = 0
        for fi in fis:
            phase_a(fi, nbf)
            nbf += fi
            if len(state) > 1:
                phase_u()
                phase_r()
        while state:
            phase_u()
            phase_r()

    nc.compile()
    return nc


# ---------------- host-side pack / unpack ----------------

_CACHE = {}


def _get_nc():
    if "nc" not in _CACHE:
        _CACHE["nc"] = build_nc(FIS)
    return _CACHE["nc"]


def _make_in_maps(x, W1, b1, W2, b2):
    x = np.asarray(x, np.float32).reshape(N)
    W1 = np.asarray(W1, np.float32)
    b1 = np.asarray(b1, np.float32)
    W2 = np.asarray(W2, np.float32)
    b2 = np.asarray(b2, np.float32).reshape(N)

    in_maps = []
    for c in range(N_CORES):
        sl = slice(c * R, (c + 1) * R)
        w1p = np.zeros((R_PAD, H), np.float16); w1p[:R] = W1[sl]
        b1p = np.zeros((R_PAD, H), np.float16); b1p[:R] = b1[sl]
        w2p = np.zeros((R_PAD, H), np.float16); w2p[:R] = W2[sl]
        xp = np.zeros(R_PAD, np.float16); xp[:R] = x[sl]
        b2p = np.zeros(R_PAD, np.float16); b2p[:R] = b2[sl]

        wa = np.empty((128, 16 * FP), np.float16)
        wb = np.empty((128, 8 * FP), np.float16)
        sm = np.empty((128, 2 * FP), np.float16)
        nbf = 0
        for fi in FIS:
            rows = slice(128 * nbf, 128 * (nbf + fi))
            # [128*fi, 8] -> [128, fi, 8] -> j-outer [128, 8, fi] -> flat
            wa[:, 16 * nbf:16 * nbf + 8 * fi] = \
                w1p[rows].reshape(128, fi, H).transpose(0, 2, 1).reshape(128, 8 * fi)
            wa[:, 16 * nbf + 8 * fi:16 * (nbf + fi)] = \
                b1p[rows].reshape(128, fi, H).transpose(0, 2, 1).reshape(128, 8 * fi)
            wb[:, 8 * nbf:8 * (nbf + fi)] = \
                w2p[rows].reshape(128, fi, H).transpose(0, 2, 1).reshape(128, 8 * fi)
            sm[:, 2 * nbf:2 * nbf + fi] = xp[rows].reshape(128, fi)
            sm[:, 2 * nbf + fi:2 * (nbf + fi)] = b2p[rows].reshape(128, fi)
            nbf += fi
        in_maps.append({"wa": wa, "wb": wb, "sm": sm})
    return in_maps


def _unpack_out(res):
    y = np.empty((N, 1), np.float32)
    for c in range(N_CORES):
        ysc = res.results[c]["ys"].reshape(128, FP)
        yflat = np.empty(R_PAD, np.float32)
        nbf = 0
        for fi in FIS:
            yflat[128 * nbf:128 * (nbf + fi)] = \
                ysc[:, nbf:nbf + fi].astype(np.float32).reshape(-1)
            nbf += fi
        y[c * R:(c + 1) * R, 0] = yflat[:R]
    return y


def _run(x, W1, b1, W2, b2, **kw):
    nc = _get_nc()
    res = run_bass_kernel_spmd(nc, _make_in_maps(x, W1, b1, W2, b2),
                               core_ids=list(range(N_CORES)), **kw)
    return _unpack_out(res), res


def kernel(x, W1, b1, W2, b2):
    y, _ = _run(x, W1, b1, W2, b2)
    return y


# revision 18
# speedup vs baseline: 1.0210x; 1.0210x over previous
"""Trainium2 Bass kernel for per-element tiny MLPs (fp16, software-pipelined).

Problem: N=4,000,000 independent 1->8->1 MLPs:
    y[i] = W2[i] @ relu(W1[i] * x[i] + b1[i]) + b2[i]

Memory-bound + DVE-bound. Sharded over 8 NeuronCores by net index (data
parallel, no communication).

Design (vs the 179-215us fp32 baseline):
  * fp16 everywhere: halves HBM traffic (52B/net in, 2B out) and gives
    tensor_tensor the 2x_1p DVE perf mode (0.52ns/elem measured, vs
    1.04 for fp32). Host-side accuracy sim: rel_l2 ~ 5e-4 (budget 2e-2).
  * hidden-dim-OUTER device layout: a weight tile is [128, 8*f] with the
    hidden index j as the outer free-dim block, so the per-net segmented
    sum is a 3-step tree of CONTIGUOUS 2x-mode tensor_tensor adds
    (tensor_reduce has no fast mode: 1 elem/cyc).
  * software pipeline: per tile, phase A = {mult x*W1, add b1} and
    phase B = {mult *W2, 3-level tree, +b2}; emitted as A_t, B_{t-1} so
    the in-order DVE stream always has B-work of the previous tile while
    ACT runs relu_t. (Without this, DVE idled ~4.5us/tile waiting on
    relu: measured 137us wall.)
  * input streams split by consumer phase: one [128, 32f] DMA carries
    w1|b1 (phase A, bufs=3 for ~2 tiles of DMA lookahead), one [128,16f]
    carries w2 (phase B), one [128, 4f] carries x|b2.
  * relu stays on the otherwise-idle ACT engine; scalar_tensor_tensor
    would fuse relu+mult but runs at 1x (no fast uop): net loss.

Per-core budget: DVE ~75us busy (32 fp16 elem/net at 2x + ~165ns/op
x 56 ops, x ~1.15 DMA-contention), DMA 27MB at ~420GB/s peak ~ 64us,
ACT relu ~27us. GPSIMD/PE idle (gpsimd steals DVE SBUF ports; PE fp32
4cyc/row and PSUM results cost 1x-mode DVE post-ops).
"""

import numpy as np
from contextlib import ExitStack

import concourse.bacc as bacc
import concourse.mybir as mybir
import concourse.tile as tile
from concourse.bass_utils import run_bass_kernel_spmd

F16 = mybir.dt.float16
AF = mybir.ActivationFunctionType
OP = mybir.AluOpType

N = 4_000_000
H = 8
N_CORES = 8
R = N // N_CORES            # 500,000 nets per core
FP = 3908                   # free-dim cols per partition: 128*3908 = 500,224
R_PAD = 128 * FP
# Ramp-up, steady-state, ramp-down tile sizes (sum = FP, all even).
FIS = [64, 256, 672, 672, 672, 672, 672, 228]
assert sum(FIS) == FP and all(f % 2 == 0 for f in FIS)


def build_nc(fis):
    fp = sum(fis)

    nc = bacc.Bacc("TRN2", target_bir_lowering=False, debug=False)

    wa = nc.dram_tensor("wa", [128, 16 * fp], F16, kind="ExternalInput")  # w1|b1
    wb = nc.dram_tensor("wb", [128, 8 * fp], F16, kind="ExternalInput")   # w2
    sm = nc.dram_tensor("sm", [128, 2 * fp], F16, kind="ExternalInput")   # x|b2
    ys = nc.dram_tensor("ys", [128, fp], F16, kind="ExternalOutput")

    with tile.TileContext(nc) as tc, ExitStack() as ctx:
        wpool = ctx.enter_context(tc.tile_pool(name="w", bufs=2))
        zpool = ctx.enter_context(tc.tile_pool(name="z", bufs=2))
        vpool = ctx.enter_context(tc.tile_pool(name="v", bufs=2))

        state = []    # tiles awaiting phase U (umults; need relu done)
        ready_r = []  # tiles awaiting phase R (reduce tree; need u)

        def phase_a(fi, nbf):
            st = vpool.tile([128, 2 * fi], F16, tag="st", bufs=3)
            nc.sync.dma_start(st[:], sm.ap()[:, 2 * nbf:2 * (nbf + fi)])
            w12 = wpool.tile([128, 16 * fi], F16, tag="w12", bufs=3)
            nc.sync.dma_start(w12[:], wa.ap()[:, 16 * nbf:16 * (nbf + fi)])
            w2t = wpool.tile([128, 8 * fi], F16, tag="w2t")
            nc.sync.dma_start(w2t[:], wb.ap()[:, 8 * nbf:8 * (nbf + fi)])

            w1 = w12[:][:, 0:8 * fi].rearrange("p (j f) -> p j f", j=H)
            b1 = w12[:][:, 8 * fi:16 * fi]
            xb = st[:][:, 0:fi].rearrange("p f -> p () f").broadcast_to([128, H, fi])

            za = zpool.tile([128, 8 * fi], F16, tag="za", bufs=3)
            nc.vector.tensor_tensor(
                za[:].rearrange("p (j f) -> p j f", j=H), xb, w1, op=OP.mult
            )
            # lo/hi halves (hidden j 0..3 / 4..7): fine-grained deps so the
            # relu halves on ACT overlap the DVE stream with <=half-relu
            # exposure regardless of scheduler order.
            zb = zpool.tile([128, 8 * fi], F16, tag="zb")
            nc.vector.tensor_tensor(zb[:][:, 0:4 * fi], za[:][:, 0:4 * fi],
                                    b1[:, 0:4 * fi], op=OP.add)
            nc.vector.tensor_tensor(zb[:][:, 4 * fi:8 * fi], za[:][:, 4 * fi:8 * fi],
                                    b1[:, 4 * fi:8 * fi], op=OP.add)
            hc = zpool.tile([128, 8 * fi], F16, tag="za", bufs=3)  # reuse ring
            nc.scalar.activation(hc[:][:, 0:4 * fi], zb[:][:, 0:4 * fi], AF.Relu)
            nc.scalar.activation(hc[:][:, 4 * fi:8 * fi], zb[:][:, 4 * fi:8 * fi],
                                 AF.Relu)
            state.append((fi, nbf, w2t, st, hc))

        def phase_u():
            fi, nbf, w2t, st, hc = state.pop(0)
            u = zpool.tile([128, 8 * fi], F16, tag="u")
            nc.vector.tensor_tensor(u[:][:, 0:4 * fi], hc[:][:, 0:4 * fi],
                                    w2t[:][:, 0:4 * fi], op=OP.mult)
            nc.vector.tensor_tensor(u[:][:, 4 * fi:8 * fi], hc[:][:, 4 * fi:8 * fi],
                                    w2t[:][:, 4 * fi:8 * fi], op=OP.mult)
            ready_r.append((fi, nbf, st, u))

        def phase_r():
            fi, nbf, st, u = ready_r.pop(0)
            b2a = st[:][:, fi:2 * fi]
            r1 = vpool.tile([128, 4 * fi], F16, tag="r1")
            nc.vector.tensor_tensor(r1[:], u[:][:, 0:4 * fi],
                                    u[:][:, 4 * fi:8 * fi], op=OP.add)
            r2 = vpool.tile([128, 2 * fi], F16, tag="r2")
            nc.vector.tensor_tensor(r2[:], r1[:][:, 0:2 * fi],
                                    r1[:][:, 2 * fi:4 * fi], op=OP.add)
            r3 = vpool.tile([128, fi], F16, tag="r3")
            nc.vector.tensor_tensor(r3[:], r2[:][:, 0:fi], r2[:][:, fi:2 * fi],
                                    op=OP.add)
            yo = vpool.tile([128, fi], F16, tag="yo")
            nc.vector.tensor_tensor(yo[:], r3[:], b2a, op=OP.add)
            nc.gpsimd.dma_start(ys.ap()[:, nbf:nbf + fi], yo[:])

        # Emission order per block: A_t, U_{t-1}, R_{t-1} - the umults of
        # tile t-1 sit AFTER the next tile's A-phase in the DVE stream, so
        # relu_{t-1} has a full A-phase (plus the prior R) of cover before
        # anything waits on it.
        nbf = 0
        for fi in fis:
            phase_a(fi, nbf)
            nbf += fi
            if len(state) > 1:
                phase_u()
                phase_r()
        while state:
            phase_u()
            phase_r()

    nc.compile()
    return nc


# ---------------- host-side pack / unpack ----------------

_CACHE = {}


def _get_nc():
    if "nc" not in _CACHE:
        _CACHE["nc"] = build_nc(FIS)
    return _CACHE["nc"]


def _make_in_maps(x, W1, b1, W2, b2):
    x = np.asarray(x, np.float32).reshape(N)
    W1 = np.asarray(W1, np.float32)
    b1 = np.asarray(b1, np.float32)
    W2 = np.asarray(W2, np.float32)
    b2 = np.asarray(b2, np.float32).reshape(N)

    in_maps = []
    for c in range(N_CORES):
        sl = slice(c * R, (c + 1) * R)
        w1p = np.zeros((R_PAD, H), np.float16); w1p[:R] = W1[sl]
        b1p = np.zeros((R_PAD, H), np.float16); b1p[:R] = b1[sl]
        w2p = np.zeros((R_PAD, H), np.float16); w2p[:R] = W2[sl]
        xp = np.zeros(R_PAD, np.float16); xp[:R] = x[sl]
        b2p = np.zeros(R_PAD, np.float16); b2p[:R] = b2[sl]

        wa = np.empty((128, 16 * FP), np.float16)
        wb = np.empty((128, 8 * FP), np.float16)
        sm = np.empty((128, 2 * FP), np.float16)
        nbf = 0
        for fi in FIS:
            rows = slice(128 * nbf, 128 * (nbf + fi))
            # [128*fi, 8] -> [128, fi, 8] -> j-outer [128, 8, fi] -> flat
            wa[:, 16 * nbf:16 * nbf + 8 * fi] = \
                w1p[rows].reshape(128, fi, H).transpose(0, 2, 1).reshape(128, 8 * fi)
            wa[:, 16 * nbf + 8 * fi:16 * (nbf + fi)] = \
                b1p[rows].reshape(128, fi, H).transpose(0, 2, 1).reshape(128, 8 * fi)
            wb[:, 8 * nbf:8 * (nbf + fi)] = \
                w2p[rows].reshape(128, fi, H).transpose(0, 2, 1).reshape(128, 8 * fi)
            sm[:, 2 * nbf:2 * nbf + fi] = xp[rows].reshape(128, fi)
            sm[:, 2 * nbf + fi:2 * (nbf + fi)] = b2p[rows].reshape(128, fi)
            nbf += fi
        in_maps.append({"wa": wa, "wb": wb, "sm": sm})
    return in_maps


def _unpack_out(res):
    y = np.empty((N, 1), np.float32)
    for c in range(N_CORES):
        ysc = res.results[c]["ys"].reshape(128, FP)
        yflat = np.empty(R_PAD, np.float32)
        nbf = 0
        for fi in FIS:
            yflat[128 * nbf:128 * (nbf + fi)] = \
                ysc[:, nbf:nbf + fi].astype(np.float32).reshape(-1)
            nbf += fi
        y[c * R:(c + 1) * R, 0] = yflat[:R]
    return y


def _run(x, W1, b1, W2, b2, **kw):
    nc = _get_nc()
    res = run_bass_kernel_spmd(nc, _make_in_maps(x, W1, b1, W2, b2),
                               core_ids=list(range(N_CORES)), **kw)
    return _unpack_out(res), res


def kernel(x, W1, b1, W2, b2):
    y, _ = _run(x, W1, b1, W2, b2)
    return y
